# revision 11
# baseline (speedup 1.0000x reference)
"""Multi-head attention (B=2, N=2048, C=768, H=12) on 8 TRN2 NeuronCores.

Sharding: core c = 4*b + g handles batch b (data parallel) and heads
3g..3g+2 (tensor parallel on H). Each core computes its 3 heads end-to-end
plus the partial projection with its 192 rows of w_proj; the host sums the
4 partials per batch and adds b_proj. No cross-device communication.

Per-core dataflow (f32 storage, fp32r TensorEngine compute):
  xt   = x[b].T                       [768, 2048]  (host layout choice)
  qkT  = [wq_h | wk_h].T @ xt chunks -> psum [128, 2048] = [qT_h; kT_h]
  kq   = partition-swapped copy        [kT_h; qT_h]  (SBUF->SBUF DMA)
  scoresT per 128-m-chunk: lhsT=kT chunk, rhs=qT  (K=64)
  expT = exp(0.125 * scoresT)          (ScalarE, [128,1024] granule)
  out_aug[65,nb] += v_aug_chunk.T @ expT  (v_aug = [v | 1] -> row 64 = colsum)
  recip broadcast via ones[1,64] matmul; outT = out_aug[0:64] * recip_bcast
  proj: out[nchunk,:] += outT_h[:, nchunk].T @ wp_h  (K=64 per head, accum)
"""

import numpy as np

import concourse.bass as bass
import concourse.mybir as mybir
import concourse.tile as tile
from concourse import bacc
from concourse.bass_utils import run_bass_kernel_spmd

F32 = mybir.dt.float32
F32R = mybir.dt.float32r
BF16 = mybir.dt.bfloat16
EXP = mybir.ActivationFunctionType.Exp
MULT = mybir.AluOpType.mult

B, N, C = 2, 2048, 768
H = 12
D = 64
HPC = 3  # heads per core
KC = 6  # contraction chunks of 128 over C
NB = 1024  # n-block for attention stage
NSUB = NB // 512
MC = N // 128  # 16 m-chunks
NCH = N // 128  # 16 row chunks of output
SCALE = D ** -0.5

_NC_CACHE = None


def build_nc():
    nc = bacc.Bacc("TRN2", target_bir_lowering=False, debug=False, num_devices=8)
    xt = nc.declare_dram_parameter("xt", [C, N], F32, isOutput=False)
    wqk = nc.declare_dram_parameter("wqk", [C, HPC * 128], F32, isOutput=False)
    wv = nc.declare_dram_parameter("wv", [C, 256], F32, isOutput=False)
    wp = nc.declare_dram_parameter("wp", [HPC * D, C], F32, isOutput=False)
    out = nc.declare_dram_parameter("out", [N, C], F32, isOutput=True)

    with tile.TileContext(nc) as tc:
        with tc.tile_pool(name="sb", bufs=1) as sb:
            # ---- load inputs -------------------------------------------------
            xt_sb = sb.tile([128, KC * N], F32, tag="xt")
            xtb = sb.tile([128, KC * N], BF16, tag="xtb")
            wqk_sb = sb.tile([128, KC * HPC * 128], F32, tag="wqk")
            wqkb = sb.tile([128, KC * HPC * 128], BF16, tag="wqkb")
            wv_sb = sb.tile([128, KC * 256], F32, tag="wv")
            wvb = sb.tile([128, KC * 256], BF16, tag="wvb")
            for kc in range(KC):
                nc.sync.dma_start(
                    wqk_sb[:, kc * HPC * 128 : (kc + 1) * HPC * 128],
                    wqk[kc * 128 : (kc + 1) * 128, :],
                )
                nc.sync.dma_start(
                    wv_sb[:, kc * 256 : (kc + 1) * 256],
                    wv[kc * 128 : (kc + 1) * 128, :],
                )
                nc.sync.dma_start(
                    xt_sb[:, kc * N : (kc + 1) * N], xt[kc * 128 : (kc + 1) * 128, :]
                )
                nc.vector.tensor_copy(
                    wqkb[:, kc * HPC * 128 : (kc + 1) * HPC * 128],
                    wqk_sb[:, kc * HPC * 128 : (kc + 1) * HPC * 128],
                )
                nc.vector.tensor_copy(
                    wvb[:, kc * 256 : (kc + 1) * 256],
                    wv_sb[:, kc * 256 : (kc + 1) * 256],
                )
                nc.vector.tensor_copy(
                    xtb[:, kc * N : (kc + 1) * N], xt_sb[:, kc * N : (kc + 1) * N]
                )
            wp01_f = sb.tile([128, C], F32, tag="wp01f")
            nc.sync.dma_start(wp01_f[:], wp[0:128, :])
            wp2_f = sb.tile([64, C], F32, tag="wp2f")
            nc.sync.dma_start(wp2_f[:], wp[128 : HPC * D, :])
            wp01 = sb.tile([128, C], BF16, tag="wp01")
            nc.vector.tensor_copy(wp01[:], wp01_f[:])
            wp2 = sb.tile([64, C], BF16, tag="wp2")
            nc.vector.tensor_copy(wp2[:], wp2_f[:])

            # constants
            ones_f = sb.tile([128, MC], F32, tag="ones_f")
            nc.vector.memset(ones_f[:], 1.0)

            # persistent activations
            v_sb = sb.tile([128, HPC * MC * 65], F32R, tag="v")
            v4 = v_sb.rearrange("p (h m w) -> p h m w", h=HPC, m=MC)
            for h in range(HPC):
                nc.vector.tensor_copy(v4[:, h, :, 64], ones_f[:, :])

            qk_sb = [sb.tile([128, N], BF16, tag=f"qk{h}", name=f"qk{h}") for h in range(HPC)]
            kq_sb = [sb.tile([128, N], BF16, tag=f"kq{h}", name=f"kq{h}") for h in range(HPC)]
            stk = sb.tile([128, N], BF16, tag="stk")
            outT1 = sb.tile([64, N], BF16, tag="outT1")
            outT2 = sb.tile([64, N], BF16, tag="outT2")

            # ---- phase 1: qkT per head; v natural ---------------------------
            with (
                tc.tile_pool(name="psum_qk", bufs=2, space="PSUM") as qkp,
                tc.tile_pool(name="psum_v", bufs=2, space="PSUM") as vp,
            ):
                for m in range(MC):
                    psv = vp.tile([128, 256], F32, tag="psv")
                    for kc in range(KC):
                        nc.tensor.matmul(
                            psv[:],
                            xtb[:, kc * N + m * 128 : kc * N + (m + 1) * 128],
                            wvb[:, kc * 256 : (kc + 1) * 256],
                            start=(kc == 0),
                            stop=(kc == KC - 1),
                        )
                    nc.vector.tensor_copy(
                        v4[:, :, m, 0:64],
                        psv.rearrange("p (h d) -> p h d", h=4)[:, 0:HPC, :],
                    )

                for h in range(HPC):
                    for half in range(2):
                        hb = half * 1024
                        ps = qkp.tile([128, 1024], F32, tag="psqk")
                        for kc in range(KC):
                            for s in range(2):
                                nc.tensor.matmul(
                                    ps[:, s * 512 : (s + 1) * 512],
                                    wqkb[
                                        :,
                                        kc * HPC * 128
                                        + h * 128 : kc * HPC * 128
                                        + (h + 1) * 128,
                                    ],
                                    xtb[
                                        :,
                                        kc * N + hb + s * 512 : kc * N + hb + (s + 1) * 512,
                                    ],
                                    start=(kc == 0),
                                    stop=(kc == KC - 1),
                                )
                        nc.vector.tensor_copy(qk_sb[h][:, hb : hb + 1024], ps[:])
                        nc.sync.dma_start(
                            kq_sb[h][0:64, hb : hb + 1024],
                            qk_sb[h][64:128, hb : hb + 1024],
                        )
                        nc.sync.dma_start(
                            kq_sb[h][64:128, hb : hb + 1024],
                            qk_sb[h][0:64, hb : hb + 1024],
                        )

            # ---- phase 2: attention per head, per n-block --------------------
            with (
                tc.tile_pool(name="psum_sc", bufs=2, space="PSUM") as scp,
                tc.tile_pool(name="psum_oa", bufs=2, space="PSUM") as oap,
                tc.tile_pool(name="dram_r", bufs=2, space="DRAM") as drp,
            ):
                def oa_mms(oa, h, m):
                    exm = ex_tiles[m % 3]
                    for s in range(NSUB):
                        nc.tensor.matmul(
                            oa[:, s * 512 : (s + 1) * 512],
                            v_sb[:, (h * MC + m) * 65 : (h * MC + m + 1) * 65],
                            exm[:, s * 512 : (s + 1) * 512],
                            start=(m == 0),
                            stop=(m == MC - 1),
                        )

                for h in range(HPC):
                    for nb in range(N // NB):
                        oa = oap.tile([65, NB], F32, tag="oa")
                        ex_tiles = [None, None, None]
                        for m in range(MC):
                            sc = scp.tile([128, NB], F32, tag="sc")
                            # two 512-halves packed on the two PE array halves
                            nc.tensor.matmul(
                                sc[:, 0:512],
                                kq_sb[h][0:64, m * 128 : (m + 1) * 128],
                                qk_sb[h][0:64, nb * NB : nb * NB + 512],
                                start=True,
                                stop=True,
                                tile_position=(0, 0),
                            )
                            nc.tensor.matmul(
                                sc[:, 512:1024],
                                qk_sb[h][64:128, m * 128 : (m + 1) * 128],
                                kq_sb[h][64:128, nb * NB + 512 : nb * NB + 1024],
                                start=True,
                                stop=True,
                                tile_position=(64, 0),
                            )
                            ex = sb.tile([128, NB], F32R, tag="ex", bufs=3)
                            nc.scalar.activation(ex[:], sc[:], EXP, scale=SCALE)
                            ex_tiles[m % 3] = ex
                            if m >= 1:
                                oa_mms(oa, h, m - 1)
                        oa_mms(oa, h, MC - 1)
                        # softmax normalization
                        cs = sb.tile([1, NB], F32, tag="cs", bufs=2)
                        nc.vector.tensor_copy(cs[:], oa[64:65, :])
                        rf = sb.tile([1, NB], F32, tag="rf", bufs=2)
                        nc.vector.reciprocal_approx_fast(out=rf[:], in_=cs[:])
                        rfd = drp.tile([1, NB], F32, tag="rfd", bufs=2)
                        nc.sync.dma_start(rfd[:], rf[:])
                        rbs = sb.tile([64, NB], F32, tag="rbs", bufs=2)
                        nc.sync.dma_start(rbs[:], rfd[:].partition_broadcast(64))
                        if h == 0:
                            mdst = stk[0:64, nb * NB : (nb + 1) * NB]
                        elif h == 1:
                            mdst = outT1[0:64, nb * NB : (nb + 1) * NB]
                        else:
                            mdst = outT2[0:64, nb * NB : (nb + 1) * NB]
                        nc.vector.tensor_tensor(
                            out=mdst,
                            in0=oa[0:64, :],
                            in1=rbs[:],
                            op=MULT,
                        )
                        if h == 1:
                            nc.sync.dma_start(
                                stk[64:128, nb * NB : (nb + 1) * NB],
                                outT1[0:64, nb * NB : (nb + 1) * NB],
                            )

            # ---- phase 3: projection (partial over this core's 192 chans) ---
            with tc.tile_pool(name="psum_pj", bufs=2, space="PSUM") as pjp:
                for m in range(NCH):
                    pp = pjp.tile([128, C], F32, tag="pp")
                    for s, w in ((0, 512), (512, 256)):
                        nc.tensor.matmul(
                            pp[:, s : s + w],
                            stk[:, m * 128 : (m + 1) * 128],
                            wp01[:, s : s + w],
                            start=True,
                            stop=False,
                        )
                    for s, w in ((0, 512), (512, 256)):
                        nc.tensor.matmul(
                            pp[:, s : s + w],
                            outT2[0:64, m * 128 : (m + 1) * 128],
                            wp2[:, s : s + w],
                            start=False,
                            stop=True,
                        )
                    ob = sb.tile([128, C], F32, tag="ob", bufs=2)
                    nc.vector.tensor_copy(ob[:], pp[:])
                    nc.sync.dma_start(out[m * 128 : (m + 1) * 128, :], ob[:])

    nc.compile()
    return nc


def get_nc():
    global _NC_CACHE
    if _NC_CACHE is None:
        _NC_CACHE = build_nc()
    return _NC_CACHE


def make_in_maps(x, w_qkv, w_proj):
    """Shard inputs for the 8 cores: core c = 4*b + g."""
    in_maps = []
    for c in range(8):
        b, g = divmod(c, 4)
        heads = [3 * g + h for h in range(HPC)]
        xt = np.ascontiguousarray(x[b].T.astype(np.float32, copy=False))
        wqk = np.empty((C, HPC * 128), dtype=np.float32)
        wv = np.zeros((C, 256), dtype=np.float32)
        for i, hh in enumerate(heads):
            wqk[:, i * 128 : i * 128 + 64] = w_qkv[:, hh * D : (hh + 1) * D]
            wqk[:, i * 128 + 64 : i * 128 + 128] = w_qkv[
                :, C + hh * D : C + (hh + 1) * D
            ]
            wv[:, i * D : (i + 1) * D] = w_qkv[:, 2 * C + hh * D : 2 * C + (hh + 1) * D]
        wp = np.ascontiguousarray(
            w_proj[g * HPC * D : (g + 1) * HPC * D, :].astype(np.float32, copy=False)
        )
        in_maps.append(
            {"xt": xt, "wqk": np.ascontiguousarray(wqk), "wv": wv, "wp": wp}
        )
    return in_maps


def run(x, w_qkv, w_proj, b_proj, trace=False):
    nc = get_nc()
    in_maps = make_in_maps(x, w_qkv, w_proj)
    res = run_bass_kernel_spmd(nc, in_maps, core_ids=list(range(8)), trace=trace)
    out = np.empty((B, N, C), dtype=np.float32)
    for b in range(B):
        acc = res.results[4 * b]["out"].astype(np.float32)
        for g in range(1, 4):
            acc = acc + res.results[4 * b + g]["out"]
        out[b] = acc + b_proj[None, :].astype(np.float32)
    return out, res


def kernel(x, w_qkv, w_proj, b_proj):
    out, _ = run(
        np.asarray(x), np.asarray(w_qkv), np.asarray(w_proj), np.asarray(b_proj)
    )
    return out


# revision 12
# speedup vs baseline: 1.0418x; 1.0418x over previous
"""Multi-head attention (B=2, N=2048, C=768, H=12) on 8 TRN2 NeuronCores.

Sharding: core c = 4*b + g handles batch b (data parallel) and heads
3g..3g+2 (tensor parallel on H). Each core computes its 3 heads end-to-end
plus the partial projection with its 192 rows of w_proj; the host sums the
4 partials per batch and adds b_proj. No cross-device communication.

Per-core dataflow (f32 storage, fp32r TensorEngine compute):
  xt   = x[b].T                       [768, 2048]  (host layout choice)
  qkT  = [wq_h | wk_h].T @ xt chunks -> psum [128, 2048] = [qT_h; kT_h]
  kq   = partition-swapped copy        [kT_h; qT_h]  (SBUF->SBUF DMA)
  scoresT per 128-m-chunk: lhsT=kT chunk, rhs=qT  (K=64)
  expT = exp(0.125 * scoresT)          (ScalarE, [128,1024] granule)
  out_aug[65,nb] += v_aug_chunk.T @ expT  (v_aug = [v | 1] -> row 64 = colsum)
  recip broadcast via ones[1,64] matmul; outT = out_aug[0:64] * recip_bcast
  proj: out[nchunk,:] += outT_h[:, nchunk].T @ wp_h  (K=64 per head, accum)
"""

import numpy as np

import concourse.bass as bass
import concourse.mybir as mybir
import concourse.tile as tile
from concourse import bacc
from concourse.bass_utils import run_bass_kernel_spmd

F32 = mybir.dt.float32
F32R = mybir.dt.float32r
BF16 = mybir.dt.bfloat16
EXP = mybir.ActivationFunctionType.Exp
MULT = mybir.AluOpType.mult

B, N, C = 2, 2048, 768
H = 12
D = 64
HPC = 3  # heads per core
KC = 6  # contraction chunks of 128 over C
NB = 1024  # n-block for attention stage
NSUB = NB // 512
MC = N // 128  # 16 m-chunks
NCH = N // 128  # 16 row chunks of output
SCALE = D ** -0.5

_NC_CACHE = None


def build_nc():
    nc = bacc.Bacc("TRN2", target_bir_lowering=False, debug=False, num_devices=8)
    xt = nc.declare_dram_parameter("xt", [C, N], F32, isOutput=False)
    wqk = nc.declare_dram_parameter("wqk", [C, HPC * 128], F32, isOutput=False)
    wv = nc.declare_dram_parameter("wv", [C, 256], F32, isOutput=False)
    wp = nc.declare_dram_parameter("wp", [HPC * D, C], F32, isOutput=False)
    out = nc.declare_dram_parameter("out", [N, C], F32, isOutput=True)

    with tile.TileContext(nc) as tc:
        with tc.tile_pool(name="sb", bufs=1) as sb:
            # ---- load inputs -------------------------------------------------
            xt_sb = sb.tile([128, KC * N], F32, tag="xt")
            xtb = sb.tile([128, KC * N], BF16, tag="xtb")
            wqk_sb = sb.tile([128, KC * HPC * 128], F32, tag="wqk")
            wqkb = sb.tile([128, KC * HPC * 128], BF16, tag="wqkb")
            wv_sb = sb.tile([128, KC * 256], F32, tag="wv")
            wvb = sb.tile([128, KC * 256], BF16, tag="wvb")
            for kc in range(KC):
                nc.sync.dma_start(
                    wqk_sb[:, kc * HPC * 128 : (kc + 1) * HPC * 128],
                    wqk[kc * 128 : (kc + 1) * 128, :],
                )
                nc.sync.dma_start(
                    wv_sb[:, kc * 256 : (kc + 1) * 256],
                    wv[kc * 128 : (kc + 1) * 128, :],
                )
                nc.sync.dma_start(
                    xt_sb[:, kc * N : (kc + 1) * N], xt[kc * 128 : (kc + 1) * 128, :]
                )
                nc.vector.tensor_copy(
                    wqkb[:, kc * HPC * 128 : (kc + 1) * HPC * 128],
                    wqk_sb[:, kc * HPC * 128 : (kc + 1) * HPC * 128],
                )
                nc.vector.tensor_copy(
                    wvb[:, kc * 256 : (kc + 1) * 256],
                    wv_sb[:, kc * 256 : (kc + 1) * 256],
                )
                nc.vector.tensor_copy(
                    xtb[:, kc * N : (kc + 1) * N], xt_sb[:, kc * N : (kc + 1) * N]
                )
            wp01_f = sb.tile([128, C], F32, tag="wp01f")
            nc.sync.dma_start(wp01_f[:], wp[0:128, :])
            wp2_f = sb.tile([64, C], F32, tag="wp2f")
            nc.sync.dma_start(wp2_f[:], wp[128 : HPC * D, :])
            wp01 = sb.tile([128, C], BF16, tag="wp01")
            nc.vector.tensor_copy(wp01[:], wp01_f[:])
            wp2 = sb.tile([64, C], BF16, tag="wp2")
            nc.vector.tensor_copy(wp2[:], wp2_f[:])

            # PE warmup: ~10us of junk matmuls to latch HAM to 2.4GHz
            junk = sb.tile([128, 512], BF16, tag="junk")
            nc.vector.memset(junk[:], 1.0)

            # constants
            ones_f = sb.tile([128, MC], F32, tag="ones_f")
            nc.vector.memset(ones_f[:], 1.0)

            # persistent activations
            v_sb = sb.tile([128, HPC * MC * 65], F32R, tag="v")
            v4 = v_sb.rearrange("p (h m w) -> p h m w", h=HPC, m=MC)
            for h in range(HPC):
                nc.vector.tensor_copy(v4[:, h, :, 64], ones_f[:, :])

            qk_sb = [sb.tile([128, N], BF16, tag=f"qk{h}", name=f"qk{h}") for h in range(HPC)]
            kq_sb = [sb.tile([128, N], BF16, tag=f"kq{h}", name=f"kq{h}") for h in range(HPC)]
            stk = sb.tile([128, N], BF16, tag="stk")
            outT1 = sb.tile([64, N], BF16, tag="outT1")
            outT2 = sb.tile([64, N], BF16, tag="outT2")

            # ---- phase 1: qkT per head; v natural ---------------------------
            with (
                tc.tile_pool(name="psum_qk", bufs=2, space="PSUM") as qkp,
                tc.tile_pool(name="psum_v", bufs=4, space="PSUM") as vp,
            ):
                for i in range(40):
                    psw = qkp.tile([128, 1024], F32, tag="psqk", name="psw")
                    nc.tensor.matmul(
                        psw[:, 0:512],
                        junk[:, 0:128],
                        junk[:],
                        start=True,
                        stop=True,
                    )
                for m in range(MC):
                    psv = vp.tile([128, 256], F32, tag="psv")
                    for kc in range(KC):
                        nc.tensor.matmul(
                            psv[:],
                            xtb[:, kc * N + m * 128 : kc * N + (m + 1) * 128],
                            wvb[:, kc * 256 : (kc + 1) * 256],
                            start=(kc == 0),
                            stop=(kc == KC - 1),
                        )
                    nc.vector.tensor_copy(
                        v4[:, :, m, 0:64],
                        psv.rearrange("p (h d) -> p h d", h=4)[:, 0:HPC, :],
                    )

                for h in range(HPC):
                    for half in range(2):
                        hb = half * 1024
                        ps = qkp.tile([128, 1024], F32, tag="psqk")
                        for kc in range(KC):
                            for s in range(2):
                                nc.tensor.matmul(
                                    ps[:, s * 512 : (s + 1) * 512],
                                    wqkb[
                                        :,
                                        kc * HPC * 128
                                        + h * 128 : kc * HPC * 128
                                        + (h + 1) * 128,
                                    ],
                                    xtb[
                                        :,
                                        kc * N + hb + s * 512 : kc * N + hb + (s + 1) * 512,
                                    ],
                                    start=(kc == 0),
                                    stop=(kc == KC - 1),
                                )
                        nc.vector.tensor_copy(qk_sb[h][:, hb : hb + 1024], ps[:])
                        nc.sync.dma_start(
                            kq_sb[h][0:64, hb : hb + 1024],
                            qk_sb[h][64:128, hb : hb + 1024],
                        )
                        nc.sync.dma_start(
                            kq_sb[h][64:128, hb : hb + 1024],
                            qk_sb[h][0:64, hb : hb + 1024],
                        )

            # ---- phase 2: attention per head, per n-block --------------------
            with (
                tc.tile_pool(name="psum_sc", bufs=2, space="PSUM") as scp,
                tc.tile_pool(name="psum_oa", bufs=2, space="PSUM") as oap,
                tc.tile_pool(name="dram_r", bufs=2, space="DRAM") as drp,
            ):
                def oa_mms(oa, h, m):
                    exm = ex_tiles[m % 3]
                    for s in range(NSUB):
                        nc.tensor.matmul(
                            oa[:, s * 512 : (s + 1) * 512],
                            v_sb[:, (h * MC + m) * 65 : (h * MC + m + 1) * 65],
                            exm[:, s * 512 : (s + 1) * 512],
                            start=(m == 0),
                            stop=(m == MC - 1),
                        )

                for h in range(HPC):
                    for nb in range(N // NB):
                        oa = oap.tile([65, NB], F32, tag="oa")
                        ex_tiles = [None, None, None]
                        for m in range(MC):
                            sc = scp.tile([128, NB], F32, tag="sc")
                            # two 512-halves packed on the two PE array halves
                            nc.tensor.matmul(
                                sc[:, 0:512],
                                kq_sb[h][0:64, m * 128 : (m + 1) * 128],
                                qk_sb[h][0:64, nb * NB : nb * NB + 512],
                                start=True,
                                stop=True,
                                tile_position=(0, 0),
                            )
                            nc.tensor.matmul(
                                sc[:, 512:1024],
                                qk_sb[h][64:128, m * 128 : (m + 1) * 128],
                                kq_sb[h][64:128, nb * NB + 512 : nb * NB + 1024],
                                start=True,
                                stop=True,
                                tile_position=(64, 0),
                            )
                            ex = sb.tile([128, NB], F32R, tag="ex", bufs=3)
                            nc.scalar.activation(ex[:], sc[:], EXP, scale=SCALE)
                            ex_tiles[m % 3] = ex
                            if m >= 1:
                                oa_mms(oa, h, m - 1)
                        oa_mms(oa, h, MC - 1)
                        # softmax normalization
                        cs = sb.tile([1, NB], F32, tag="cs", bufs=2)
                        nc.vector.tensor_copy(cs[:], oa[64:65, :])
                        rf = sb.tile([1, NB], F32, tag="rf", bufs=2)
                        nc.vector.reciprocal_approx_fast(out=rf[:], in_=cs[:])
                        rfd = drp.tile([1, NB], F32, tag="rfd", bufs=2)
                        nc.sync.dma_start(rfd[:], rf[:])
                        rbs = sb.tile([64, NB], F32, tag="rbs", bufs=2)
                        nc.sync.dma_start(rbs[:], rfd[:].partition_broadcast(64))
                        if h == 0:
                            mdst = stk[0:64, nb * NB : (nb + 1) * NB]
                        elif h == 1:
                            mdst = outT1[0:64, nb * NB : (nb + 1) * NB]
                        else:
                            mdst = outT2[0:64, nb * NB : (nb + 1) * NB]
                        nc.vector.tensor_tensor(
                            out=mdst,
                            in0=oa[0:64, :],
                            in1=rbs[:],
                            op=MULT,
                        )
                        if h == 1:
                            nc.sync.dma_start(
                                stk[64:128, nb * NB : (nb + 1) * NB],
                                outT1[0:64, nb * NB : (nb + 1) * NB],
                            )

            # ---- phase 3: projection (partial over this core's 192 chans) ---
            with tc.tile_pool(name="psum_pj", bufs=2, space="PSUM") as pjp:
                for m in range(NCH):
                    pp = pjp.tile([128, C], F32, tag="pp")
                    for s, w in ((0, 512), (512, 256)):
                        nc.tensor.matmul(
                            pp[:, s : s + w],
                            stk[:, m * 128 : (m + 1) * 128],
                            wp01[:, s : s + w],
                            start=True,
                            stop=False,
                        )
                    for s, w in ((0, 512), (512, 256)):
                        nc.tensor.matmul(
                            pp[:, s : s + w],
                            outT2[0:64, m * 128 : (m + 1) * 128],
                            wp2[:, s : s + w],
                            start=False,
                            stop=True,
                        )
                    ob = sb.tile([128, C], F32, tag="ob", bufs=3)
                    nc.vector.tensor_copy(ob[:, 0:384], pp[:, 0:384])
                    nc.scalar.copy(out=ob[:, 384:768], in_=pp[:, 384:768])
                    nc.sync.dma_start(out[m * 128 : (m + 1) * 128, :], ob[:])

    nc.compile()
    return nc


def get_nc():
    global _NC_CACHE
    if _NC_CACHE is None:
        _NC_CACHE = build_nc()
    return _NC_CACHE


def make_in_maps(x, w_qkv, w_proj):
    """Shard inputs for the 8 cores: core c = 4*b + g."""
    in_maps = []
    for c in range(8):
        b, g = divmod(c, 4)
        heads = [3 * g + h for h in range(HPC)]
        xt = np.ascontiguousarray(x[b].T.astype(np.float32, copy=False))
        wqk = np.empty((C, HPC * 128), dtype=np.float32)
        wv = np.zeros((C, 256), dtype=np.float32)
        for i, hh in enumerate(heads):
            wqk[:, i * 128 : i * 128 + 64] = w_qkv[:, hh * D : (hh + 1) * D]
            wqk[:, i * 128 + 64 : i * 128 + 128] = w_qkv[
                :, C + hh * D : C + (hh + 1) * D
            ]
            wv[:, i * D : (i + 1) * D] = w_qkv[:, 2 * C + hh * D : 2 * C + (hh + 1) * D]
        wp = np.ascontiguousarray(
            w_proj[g * HPC * D : (g + 1) * HPC * D, :].astype(np.float32, copy=False)
        )
        in_maps.append(
            {"xt": xt, "wqk": np.ascontiguousarray(wqk), "wv": wv, "wp": wp}
        )
    return in_maps


def run(x, w_qkv, w_proj, b_proj, trace=False):
    nc = get_nc()
    in_maps = make_in_maps(x, w_qkv, w_proj)
    res = run_bass_kernel_spmd(nc, in_maps, core_ids=list(range(8)), trace=trace)
    out = np.empty((B, N, C), dtype=np.float32)
    for b in range(B):
        acc = res.results[4 * b]["out"].astype(np.float32)
        for g in range(1, 4):
            acc = acc + res.results[4 * b + g]["out"]
        out[b] = acc + b_proj[None, :].astype(np.float32)
    return out, res


def kernel(x, w_qkv, w_proj, b_proj):
    out, _ = run(
        np.asarray(x), np.asarray(w_qkv), np.asarray(w_proj), np.asarray(b_proj)
    )
    return out


# revision 13
# speedup vs baseline: 1.0677x; 1.0248x over previous
"""Multi-head attention (B=2, N=2048, C=768, H=12) on 8 TRN2 NeuronCores.

Sharding: core c = 4*b + g handles batch b (data parallel) and heads
3g..3g+2 (tensor parallel on H). Each core computes its 3 heads end-to-end
plus the partial projection with its 192 rows of w_proj; the host sums the
4 partials per batch and adds b_proj. No cross-device communication.

Per-core dataflow (f32 storage, fp32r TensorEngine compute):
  xt   = x[b].T                       [768, 2048]  (host layout choice)
  qkT  = [wq_h | wk_h].T @ xt chunks -> psum [128, 2048] = [qT_h; kT_h]
  kq   = partition-swapped copy        [kT_h; qT_h]  (SBUF->SBUF DMA)
  scoresT per 128-m-chunk: lhsT=kT chunk, rhs=qT  (K=64)
  expT = exp(0.125 * scoresT)          (ScalarE, [128,1024] granule)
  out_aug[65,nb] += v_aug_chunk.T @ expT  (v_aug = [v | 1] -> row 64 = colsum)
  recip broadcast via ones[1,64] matmul; outT = out_aug[0:64] * recip_bcast
  proj: out[nchunk,:] += outT_h[:, nchunk].T @ wp_h  (K=64 per head, accum)
"""

import numpy as np

import concourse.bass as bass
import concourse.mybir as mybir
import concourse.tile as tile
from concourse import bacc
from concourse.bass_utils import run_bass_kernel_spmd

F32 = mybir.dt.float32
F32R = mybir.dt.float32r
BF16 = mybir.dt.bfloat16
EXP = mybir.ActivationFunctionType.Exp
MULT = mybir.AluOpType.mult

B, N, C = 2, 2048, 768
H = 12
D = 64
HPC = 3  # heads per core
KC = 6  # contraction chunks of 128 over C
NB = 1024  # n-block for attention stage
NSUB = NB // 512
MC = N // 128  # 16 m-chunks
NCH = N // 128  # 16 row chunks of output
SCALE = D ** -0.5

_NC_CACHE = None


def build_nc():
    nc = bacc.Bacc("TRN2", target_bir_lowering=False, debug=False, num_devices=8)
    xt = nc.declare_dram_parameter("xt", [C, N], F32, isOutput=False)
    wqk = nc.declare_dram_parameter("wqk", [C, HPC * 128], F32, isOutput=False)
    wv = nc.declare_dram_parameter("wv", [C, 256], F32, isOutput=False)
    wp = nc.declare_dram_parameter("wp", [HPC * D, C], F32, isOutput=False)
    out = nc.declare_dram_parameter("out", [N, C], F32, isOutput=True)

    with tile.TileContext(nc) as tc:
        with tc.tile_pool(name="sb", bufs=1) as sb:
            # ---- load inputs -------------------------------------------------
            xt_sb = sb.tile([128, KC * N], F32, tag="xt")
            xtb = sb.tile([128, KC * N], BF16, tag="xtb")
            wqk_sb = sb.tile([128, KC * HPC * 128], F32, tag="wqk")
            wqkb = sb.tile([128, KC * HPC * 128], BF16, tag="wqkb")
            wv_sb = sb.tile([128, KC * 256], F32, tag="wv")
            wvb = sb.tile([128, KC * 256], BF16, tag="wvb")
            for kc in range(KC):
                nc.sync.dma_start(
                    wqk_sb[:, kc * HPC * 128 : (kc + 1) * HPC * 128],
                    wqk[kc * 128 : (kc + 1) * 128, :],
                )
                nc.sync.dma_start(
                    wv_sb[:, kc * 256 : (kc + 1) * 256],
                    wv[kc * 128 : (kc + 1) * 128, :],
                )
                nc.sync.dma_start(
                    xt_sb[:, kc * N : (kc + 1) * N], xt[kc * 128 : (kc + 1) * 128, :]
                )
                nc.vector.tensor_copy(
                    wqkb[:, kc * HPC * 128 : (kc + 1) * HPC * 128],
                    wqk_sb[:, kc * HPC * 128 : (kc + 1) * HPC * 128],
                )
                nc.vector.tensor_copy(
                    wvb[:, kc * 256 : (kc + 1) * 256],
                    wv_sb[:, kc * 256 : (kc + 1) * 256],
                )
                nc.vector.tensor_copy(
                    xtb[:, kc * N : (kc + 1) * N], xt_sb[:, kc * N : (kc + 1) * N]
                )
            wp01_f = sb.tile([128, C], F32, tag="wp01f")
            nc.sync.dma_start(wp01_f[:], wp[0:128, :])
            wp2_f = sb.tile([64, C], F32, tag="wp2f")
            nc.sync.dma_start(wp2_f[:], wp[128 : HPC * D, :])
            wp01 = sb.tile([128, C], BF16, tag="wp01")
            nc.vector.tensor_copy(wp01[:], wp01_f[:])
            wp2 = sb.tile([64, C], BF16, tag="wp2")
            nc.vector.tensor_copy(wp2[:], wp2_f[:])

            # PE warmup: ~10us of junk matmuls to latch HAM to 2.4GHz
            junk = sb.tile([128, 512], BF16, tag="junk")
            nc.vector.memset(junk[:], 1.0)

            # constants
            ones_f = sb.tile([128, MC], F32, tag="ones_f")
            nc.vector.memset(ones_f[:], 1.0)

            # persistent activations
            v_sb = sb.tile([128, HPC * MC * 65], F32R, tag="v")
            v4 = v_sb.rearrange("p (h m w) -> p h m w", h=HPC, m=MC)
            for h in range(HPC):
                nc.vector.tensor_copy(v4[:, h, :, 64], ones_f[:, :])

            qk_sb = [sb.tile([128, N], BF16, tag=f"qk{h}", name=f"qk{h}") for h in range(HPC)]
            kq_sb = [sb.tile([128, N], BF16, tag=f"kq{h}", name=f"kq{h}") for h in range(HPC)]
            stk = sb.tile([128, N], BF16, tag="stk")
            outT1 = sb.tile([64, N], BF16, tag="outT1")
            outT2 = sb.tile([64, N], BF16, tag="outT2")

            # ---- phase 1: qkT per head; v natural ---------------------------
            with (
                tc.tile_pool(name="psum_qk", bufs=2, space="PSUM") as qkp,
                tc.tile_pool(name="psum_v", bufs=4, space="PSUM") as vp,
            ):
                for i in range(40):
                    psw = qkp.tile([128, 1024], F32, tag="psqk", name="psw")
                    nc.tensor.matmul(
                        psw[:, 0:512],
                        junk[:, 0:128],
                        junk[:],
                        start=True,
                        stop=True,
                    )
                for h in range(HPC):
                    for half in range(2):
                        hb = half * 1024
                        ps = qkp.tile([128, 1024], F32, tag="psqk")
                        for kc in range(KC):
                            for s in range(2):
                                nc.tensor.matmul(
                                    ps[:, s * 512 : (s + 1) * 512],
                                    wqkb[
                                        :,
                                        kc * HPC * 128
                                        + h * 128 : kc * HPC * 128
                                        + (h + 1) * 128,
                                    ],
                                    xtb[
                                        :,
                                        kc * N + hb + s * 512 : kc * N + hb + (s + 1) * 512,
                                    ],
                                    start=(kc == 0),
                                    stop=(kc == KC - 1),
                                )
                        if half == 0:
                            nc.vector.tensor_copy(qk_sb[h][:, hb : hb + 1024], ps[:])
                        else:
                            nc.scalar.copy(out=qk_sb[h][:, hb : hb + 1024], in_=ps[:])
                        nc.sync.dma_start(
                            kq_sb[h][0:64, hb : hb + 1024],
                            qk_sb[h][64:128, hb : hb + 1024],
                        )
                        nc.sync.dma_start(
                            kq_sb[h][64:128, hb : hb + 1024],
                            qk_sb[h][0:64, hb : hb + 1024],
                        )

                for m in range(MC):
                    psv = vp.tile([128, 256], F32, tag="psv")
                    for kc in range(KC):
                        nc.tensor.matmul(
                            psv[:],
                            xtb[:, kc * N + m * 128 : kc * N + (m + 1) * 128],
                            wvb[:, kc * 256 : (kc + 1) * 256],
                            start=(kc == 0),
                            stop=(kc == KC - 1),
                        )
                    nc.scalar.copy(
                        out=v4[:, :, m, 0:64],
                        in_=psv.rearrange("p (h d) -> p h d", h=4)[:, 0:HPC, :],
                    )

            # ---- phase 2: attention per head, per n-block --------------------
            with (
                tc.tile_pool(name="psum_sc", bufs=2, space="PSUM") as scp,
                tc.tile_pool(name="psum_oa", bufs=2, space="PSUM") as oap,
                tc.tile_pool(name="dram_r", bufs=2, space="DRAM") as drp,
            ):
                def oa_mms(oa, h, m):
                    exm = ex_tiles[m % 3]
                    for s in range(NSUB):
                        nc.tensor.matmul(
                            oa[:, s * 512 : (s + 1) * 512],
                            v_sb[:, (h * MC + m) * 65 : (h * MC + m + 1) * 65],
                            exm[:, s * 512 : (s + 1) * 512],
                            start=(m == 0),
                            stop=(m == MC - 1),
                        )

                for h in range(HPC):
                    for nb in range(N // NB):
                        oa = oap.tile([65, NB], F32, tag="oa")
                        ex_tiles = [None, None, None]
                        for m in range(MC):
                            sc = scp.tile([128, NB], F32, tag="sc")
                            # two 512-halves packed on the two PE array halves
                            nc.tensor.matmul(
                                sc[:, 0:512],
                                kq_sb[h][0:64, m * 128 : (m + 1) * 128],
                                qk_sb[h][0:64, nb * NB : nb * NB + 512],
                                start=True,
                                stop=True,
                                tile_position=(0, 0),
                            )
                            nc.tensor.matmul(
                                sc[:, 512:1024],
                                qk_sb[h][64:128, m * 128 : (m + 1) * 128],
                                kq_sb[h][64:128, nb * NB + 512 : nb * NB + 1024],
                                start=True,
                                stop=True,
                                tile_position=(64, 0),
                            )
                            ex = sb.tile([128, NB], F32R, tag="ex", bufs=3)
                            nc.scalar.activation(ex[:], sc[:], EXP, scale=SCALE)
                            ex_tiles[m % 3] = ex
                            if m >= 1:
                                oa_mms(oa, h, m - 1)
                        oa_mms(oa, h, MC - 1)
                        # softmax normalization
                        cs = sb.tile([1, NB], F32, tag="cs", bufs=2)
                        nc.vector.tensor_copy(cs[:], oa[64:65, :])
                        rf = sb.tile([1, NB], F32, tag="rf", bufs=2)
                        nc.vector.reciprocal_approx_fast(out=rf[:], in_=cs[:])
                        rfd = drp.tile([1, NB], F32, tag="rfd", bufs=2)
                        nc.sync.dma_start(rfd[:], rf[:])
                        rbs = sb.tile([64, NB], F32, tag="rbs", bufs=2)
                        nc.sync.dma_start(rbs[:], rfd[:].partition_broadcast(64))
                        if h == 0:
                            mdst = stk[0:64, nb * NB : (nb + 1) * NB]
                        elif h == 1:
                            mdst = outT1[0:64, nb * NB : (nb + 1) * NB]
                        else:
                            mdst = outT2[0:64, nb * NB : (nb + 1) * NB]
                        nc.vector.tensor_tensor(
                            out=mdst,
                            in0=oa[0:64, :],
                            in1=rbs[:],
                            op=MULT,
                        )
                        if h == 1:
                            nc.sync.dma_start(
                                stk[64:128, nb * NB : (nb + 1) * NB],
                                outT1[0:64, nb * NB : (nb + 1) * NB],
                            )

            # ---- phase 3: projection (partial over this core's 192 chans) ---
            with tc.tile_pool(name="psum_pj", bufs=2, space="PSUM") as pjp:
                for m in range(NCH):
                    pp = pjp.tile([128, C], F32, tag="pp")
                    for s, w in ((0, 512), (512, 256)):
                        nc.tensor.matmul(
                            pp[:, s : s + w],
                            stk[:, m * 128 : (m + 1) * 128],
                            wp01[:, s : s + w],
                            start=True,
                            stop=False,
                        )
                    for s, w in ((0, 512), (512, 256)):
                        nc.tensor.matmul(
                            pp[:, s : s + w],
                            outT2[0:64, m * 128 : (m + 1) * 128],
                            wp2[:, s : s + w],
                            start=False,
                            stop=True,
                        )
                    ob = sb.tile([128, C], F32, tag="ob", bufs=3)
                    nc.vector.tensor_copy(ob[:, 0:384], pp[:, 0:384])
                    nc.scalar.copy(out=ob[:, 384:768], in_=pp[:, 384:768])
                    nc.sync.dma_start(out[m * 128 : (m + 1) * 128, :], ob[:])

    nc.compile()
    return nc


def get_nc():
    global _NC_CACHE
    if _NC_CACHE is None:
        _NC_CACHE = build_nc()
    return _NC_CACHE


def make_in_maps(x, w_qkv, w_proj):
    """Shard inputs for the 8 cores: core c = 4*b + g."""
    in_maps = []
    for c in range(8):
        b, g = divmod(c, 4)
        heads = [3 * g + h for h in range(HPC)]
        xt = np.ascontiguousarray(x[b].T.astype(np.float32, copy=False))
        wqk = np.empty((C, HPC * 128), dtype=np.float32)
        wv = np.zeros((C, 256), dtype=np.float32)
        for i, hh in enumerate(heads):
            wqk[:, i * 128 : i * 128 + 64] = w_qkv[:, hh * D : (hh + 1) * D]
            wqk[:, i * 128 + 64 : i * 128 + 128] = w_qkv[
                :, C + hh * D : C + (hh + 1) * D
            ]
            wv[:, i * D : (i + 1) * D] = w_qkv[:, 2 * C + hh * D : 2 * C + (hh + 1) * D]
        wp = np.ascontiguousarray(
            w_proj[g * HPC * D : (g + 1) * HPC * D, :].astype(np.float32, copy=False)
        )
        in_maps.append(
            {"xt": xt, "wqk": np.ascontiguousarray(wqk), "wv": wv, "wp": wp}
        )
    return in_maps


def run(x, w_qkv, w_proj, b_proj, trace=False):
    nc = get_nc()
    in_maps = make_in_maps(x, w_qkv, w_proj)
    res = run_bass_kernel_spmd(nc, in_maps, core_ids=list(range(8)), trace=trace)
    out = np.empty((B, N, C), dtype=np.float32)
    for b in range(B):
        acc = res.results[4 * b]["out"].astype(np.float32)
        for g in range(1, 4):
            acc = acc + res.results[4 * b + g]["out"]
        out[b] = acc + b_proj[None, :].astype(np.float32)
    return out, res


def kernel(x, w_qkv, w_proj, b_proj):
    out, _ = run(
        np.asarray(x), np.asarray(w_qkv), np.asarray(w_proj), np.asarray(b_proj)
    )
    return out


# revision 14
# speedup vs baseline: 1.0958x; 1.0264x over previous
"""Multi-head attention (B=2, N=2048, C=768, H=12) on 8 TRN2 NeuronCores.

Sharding: core c = 4*b + g handles batch b (data parallel) and heads
3g..3g+2 (tensor parallel on H). Each core computes its 3 heads end-to-end
plus the partial projection with its 192 rows of w_proj; the host sums the
4 partials per batch and adds b_proj. No cross-device communication.

Per-core dataflow (f32 storage, fp32r TensorEngine compute):
  xt   = x[b].T                       [768, 2048]  (host layout choice)
  qkT  = [wq_h | wk_h].T @ xt chunks -> psum [128, 2048] = [qT_h; kT_h]
  kq   = partition-swapped copy        [kT_h; qT_h]  (SBUF->SBUF DMA)
  scoresT per 128-m-chunk: lhsT=kT chunk, rhs=qT  (K=64)
  expT = exp(0.125 * scoresT)          (ScalarE, [128,1024] granule)
  out_aug[65,nb] += v_aug_chunk.T @ expT  (v_aug = [v | 1] -> row 64 = colsum)
  recip broadcast via ones[1,64] matmul; outT = out_aug[0:64] * recip_bcast
  proj: out[nchunk,:] += outT_h[:, nchunk].T @ wp_h  (K=64 per head, accum)
"""

import numpy as np

import concourse.bass as bass
import concourse.mybir as mybir
import concourse.tile as tile
from concourse import bacc
from concourse.bass_utils import run_bass_kernel_spmd

F32 = mybir.dt.float32
F32R = mybir.dt.float32r
BF16 = mybir.dt.bfloat16
EXP = mybir.ActivationFunctionType.Exp
MULT = mybir.AluOpType.mult

B, N, C = 2, 2048, 768
H = 12
D = 64
HPC = 3  # heads per core
KC = 6  # contraction chunks of 128 over C
NB = 1024  # n-block for attention stage
NSUB = NB // 512
MC = N // 128  # 16 m-chunks
NCH = N // 128  # 16 row chunks of output
SCALE = D ** -0.5

_NC_CACHE = None


def build_nc():
    nc = bacc.Bacc("TRN2", target_bir_lowering=False, debug=False, num_devices=8)
    xt = nc.declare_dram_parameter("xt", [C, N], F32, isOutput=False)
    wqk = nc.declare_dram_parameter("wqk", [C, HPC * 128], F32, isOutput=False)
    wv = nc.declare_dram_parameter("wv", [C, 256], F32, isOutput=False)
    wp = nc.declare_dram_parameter("wp", [HPC * D, C], F32, isOutput=False)
    out = nc.declare_dram_parameter("out", [N, C], F32, isOutput=True)

    with tile.TileContext(nc) as tc:
        with tc.tile_pool(name="sb", bufs=1) as sb:
            # ---- load inputs -------------------------------------------------
            xt_sb = sb.tile([128, KC * N], F32, tag="xt")
            xtb = sb.tile([128, KC * N], BF16, tag="xtb")
            wqk_sb = sb.tile([128, KC * HPC * 128], F32, tag="wqk")
            wqkb = sb.tile([128, KC * HPC * 128], BF16, tag="wqkb")
            wv_sb = sb.tile([128, KC * 256], F32, tag="wv")
            wvb = sb.tile([128, KC * 256], BF16, tag="wvb")
            for kc in range(KC):
                nc.sync.dma_start(
                    wqk_sb[:, kc * HPC * 128 : (kc + 1) * HPC * 128],
                    wqk[kc * 128 : (kc + 1) * 128, :],
                )
                nc.sync.dma_start(
                    wv_sb[:, kc * 256 : (kc + 1) * 256],
                    wv[kc * 128 : (kc + 1) * 128, :],
                )
                nc.sync.dma_start(
                    xt_sb[:, kc * N : (kc + 1) * N], xt[kc * 128 : (kc + 1) * 128, :]
                )
                nc.vector.tensor_copy(
                    wqkb[:, kc * HPC * 128 : (kc + 1) * HPC * 128],
                    wqk_sb[:, kc * HPC * 128 : (kc + 1) * HPC * 128],
                )
                nc.vector.tensor_copy(
                    wvb[:, kc * 256 : (kc + 1) * 256],
                    wv_sb[:, kc * 256 : (kc + 1) * 256],
                )
                nc.vector.tensor_copy(
                    xtb[:, kc * N : (kc + 1) * N], xt_sb[:, kc * N : (kc + 1) * N]
                )
            wp01_f = sb.tile([128, C], F32, tag="wp01f")
            nc.sync.dma_start(wp01_f[:], wp[0:128, :])
            wp2_f = sb.tile([64, C], F32, tag="wp2f")
            nc.sync.dma_start(wp2_f[:], wp[128 : HPC * D, :])
            wp01 = sb.tile([128, C], BF16, tag="wp01")
            nc.vector.tensor_copy(wp01[:], wp01_f[:])
            wp2 = sb.tile([64, C], BF16, tag="wp2")
            nc.vector.tensor_copy(wp2[:], wp2_f[:])

            # PE warmup: ~10us of junk matmuls to latch HAM to 2.4GHz
            junk = sb.tile([128, 512], BF16, tag="junk")
            nc.vector.memset(junk[:], 1.0)

            # constants
            ones_f = sb.tile([128, MC], F32, tag="ones_f")
            nc.vector.memset(ones_f[:], 1.0)

            # persistent activations
            v_sb = sb.tile([128, HPC * MC * 65], F32R, tag="v")
            v4 = v_sb.rearrange("p (h m w) -> p h m w", h=HPC, m=MC)
            for h in range(HPC):
                nc.vector.tensor_copy(v4[:, h, :, 64], ones_f[:, :])

            qk_sb = [sb.tile([128, N], BF16, tag=f"qk{h}", name=f"qk{h}") for h in range(HPC)]
            kq_sb = [sb.tile([128, N], BF16, tag=f"kq{h}", name=f"kq{h}") for h in range(HPC)]
            stk = sb.tile([128, N], BF16, tag="stk")
            outT1 = sb.tile([64, N], BF16, tag="outT1")
            outT2 = sb.tile([64, N], BF16, tag="outT2")

            # ---- phase 1: qkT per head; v natural ---------------------------
            with (
                tc.tile_pool(name="psum_qk", bufs=2, space="PSUM") as qkp,
                tc.tile_pool(name="psum_v", bufs=4, space="PSUM") as vp,
            ):
                for i in range(16):
                    psw = qkp.tile([128, 1024], F32, tag="psqk", name="psw")
                    nc.tensor.matmul(
                        psw[:, 0:512],
                        junk[:, 0:128],
                        junk[:],
                        start=True,
                        stop=True,
                    )
                for h in range(HPC):
                    for half in range(2):
                        hb = half * 1024
                        ps = qkp.tile([128, 1024], F32, tag="psqk")
                        for kc in range(KC):
                            for s in range(2):
                                nc.tensor.matmul(
                                    ps[:, s * 512 : (s + 1) * 512],
                                    wqkb[
                                        :,
                                        kc * HPC * 128
                                        + h * 128 : kc * HPC * 128
                                        + (h + 1) * 128,
                                    ],
                                    xtb[
                                        :,
                                        kc * N + hb + s * 512 : kc * N + hb + (s + 1) * 512,
                                    ],
                                    start=(kc == 0),
                                    stop=(kc == KC - 1),
                                )
                        if half == 0:
                            nc.vector.tensor_copy(qk_sb[h][:, hb : hb + 1024], ps[:])
                        else:
                            nc.scalar.copy(out=qk_sb[h][:, hb : hb + 1024], in_=ps[:])
                        nc.sync.dma_start(
                            kq_sb[h][0:64, hb : hb + 1024],
                            qk_sb[h][64:128, hb : hb + 1024],
                        )
                        nc.sync.dma_start(
                            kq_sb[h][64:128, hb : hb + 1024],
                            qk_sb[h][0:64, hb : hb + 1024],
                        )

                for m in range(MC):
                    psv = vp.tile([128, 256], F32, tag="psv")
                    for kc in range(KC):
                        nc.tensor.matmul(
                            psv[:],
                            xtb[:, kc * N + m * 128 : kc * N + (m + 1) * 128],
                            wvb[:, kc * 256 : (kc + 1) * 256],
                            start=(kc == 0),
                            stop=(kc == KC - 1),
                        )
                    nc.scalar.copy(
                        out=v4[:, :, m, 0:64],
                        in_=psv.rearrange("p (h d) -> p h d", h=4)[:, 0:HPC, :],
                    )

            # ---- phase 2: attention per head, per n-block --------------------
            with (
                tc.tile_pool(name="psum_sc", bufs=2, space="PSUM") as scp,
                tc.tile_pool(name="psum_oa", bufs=2, space="PSUM") as oap,
                tc.tile_pool(name="dram_r", bufs=2, space="DRAM") as drp,
            ):
                def oa_mms(oa, h, m):
                    exm = ex_tiles[m % 3]
                    for s in range(NSUB):
                        nc.tensor.matmul(
                            oa[:, s * 512 : (s + 1) * 512],
                            v_sb[:, (h * MC + m) * 65 : (h * MC + m + 1) * 65],
                            exm[:, s * 512 : (s + 1) * 512],
                            start=(m == 0),
                            stop=(m == MC - 1),
                        )

                for h in range(HPC):
                    for nb in range(N // NB):
                        oa = oap.tile([65, NB], F32, tag="oa")
                        ex_tiles = [None, None, None]
                        for m in range(MC):
                            sc = scp.tile([128, NB], F32, tag="sc")
                            # two 512-halves packed on the two PE array halves
                            nc.tensor.matmul(
                                sc[:, 0:512],
                                kq_sb[h][0:64, m * 128 : (m + 1) * 128],
                                qk_sb[h][0:64, nb * NB : nb * NB + 512],
                                start=True,
                                stop=True,
                                tile_position=(0, 0),
                            )
                            nc.tensor.matmul(
                                sc[:, 512:1024],
                                qk_sb[h][64:128, m * 128 : (m + 1) * 128],
                                kq_sb[h][64:128, nb * NB + 512 : nb * NB + 1024],
                                start=True,
                                stop=True,
                                tile_position=(64, 0),
                            )
                            ex = sb.tile([128, NB], F32R, tag="ex", bufs=3)
                            nc.scalar.activation(ex[:], sc[:], EXP, scale=SCALE)
                            ex_tiles[m % 3] = ex
                            if m >= 1:
                                oa_mms(oa, h, m - 1)
                        oa_mms(oa, h, MC - 1)
                        # softmax normalization
                        cs = sb.tile([1, NB], F32, tag="cs", bufs=2)
                        nc.vector.tensor_copy(cs[:], oa[64:65, :])
                        rf = sb.tile([1, NB], F32, tag="rf", bufs=2)
                        nc.vector.reciprocal_approx_fast(out=rf[:], in_=cs[:])
                        rfd = drp.tile([1, NB], F32, tag="rfd", bufs=2)
                        nc.sync.dma_start(rfd[:], rf[:])
                        rbs = sb.tile([64, NB], F32, tag="rbs", bufs=2)
                        nc.sync.dma_start(rbs[:], rfd[:].partition_broadcast(64))
                        if h == 0:
                            mdst = stk[0:64, nb * NB : (nb + 1) * NB]
                        elif h == 1:
                            mdst = outT1[0:64, nb * NB : (nb + 1) * NB]
                        else:
                            mdst = outT2[0:64, nb * NB : (nb + 1) * NB]
                        nc.vector.tensor_tensor(
                            out=mdst,
                            in0=oa[0:64, :],
                            in1=rbs[:],
                            op=MULT,
                        )
                        if h == 1:
                            nc.sync.dma_start(
                                stk[64:128, nb * NB : (nb + 1) * NB],
                                outT1[0:64, nb * NB : (nb + 1) * NB],
                            )

            # ---- phase 3: projection (partial over this core's 192 chans) ---
            with tc.tile_pool(name="psum_pj", bufs=2, space="PSUM") as pjp:
                for m in range(NCH):
                    pp = pjp.tile([128, C], F32, tag="pp")
                    for s, w in ((0, 512), (512, 256)):
                        nc.tensor.matmul(
                            pp[:, s : s + w],
                            stk[:, m * 128 : (m + 1) * 128],
                            wp01[:, s : s + w],
                            start=True,
                            stop=False,
                        )
                    for s, w in ((0, 512), (512, 256)):
                        nc.tensor.matmul(
                            pp[:, s : s + w],
                            outT2[0:64, m * 128 : (m + 1) * 128],
                            wp2[:, s : s + w],
                            start=False,
                            stop=True,
                        )
                    ob = sb.tile([128, C], F32, tag="ob", bufs=4)
                    nc.vector.tensor_copy(ob[:, 0:384], pp[:, 0:384])
                    nc.scalar.copy(out=ob[:, 384:768], in_=pp[:, 384:768])
                    nc.sync.dma_start(out[m * 128 : (m + 1) * 128, 0:384], ob[:, 0:384])
                    nc.sync.dma_start(out[m * 128 : (m + 1) * 128, 384:768], ob[:, 384:768])

    nc.compile()
    return nc


def get_nc():
    global _NC_CACHE
    if _NC_CACHE is None:
        _NC_CACHE = build_nc()
    return _NC_CACHE


def make_in_maps(x, w_qkv, w_proj):
    """Shard inputs for the 8 cores: core c = 4*b + g."""
    in_maps = []
    for c in range(8):
        b, g = divmod(c, 4)
        heads = [3 * g + h for h in range(HPC)]
        xt = np.ascontiguousarray(x[b].T.astype(np.float32, copy=False))
        wqk = np.empty((C, HPC * 128), dtype=np.float32)
        wv = np.zeros((C, 256), dtype=np.float32)
        for i, hh in enumerate(heads):
            wqk[:, i * 128 : i * 128 + 64] = w_qkv[:, hh * D : (hh + 1) * D]
            wqk[:, i * 128 + 64 : i * 128 + 128] = w_qkv[
                :, C + hh * D : C + (hh + 1) * D
            ]
            wv[:, i * D : (i + 1) * D] = w_qkv[:, 2 * C + hh * D : 2 * C + (hh + 1) * D]
        wp = np.ascontiguousarray(
            w_proj[g * HPC * D : (g + 1) * HPC * D, :].astype(np.float32, copy=False)
        )
        in_maps.append(
            {"xt": xt, "wqk": np.ascontiguousarray(wqk), "wv": wv, "wp": wp}
        )
    return in_maps


def run(x, w_qkv, w_proj, b_proj, trace=False):
    nc = get_nc()
    in_maps = make_in_maps(x, w_qkv, w_proj)
    res = run_bass_kernel_spmd(nc, in_maps, core_ids=list(range(8)), trace=trace)
    out = np.empty((B, N, C), dtype=np.float32)
    for b in range(B):
        acc = res.results[4 * b]["out"].astype(np.float32)
        for g in range(1, 4):
            acc = acc + res.results[4 * b + g]["out"]
        out[b] = acc + b_proj[None, :].astype(np.float32)
    return out, res


def kernel(x, w_qkv, w_proj, b_proj):
    out, _ = run(
        np.asarray(x), np.asarray(w_qkv), np.asarray(w_proj), np.asarray(b_proj)
    )
    return out


# revision 15
# speedup vs baseline: 1.1544x; 1.0535x over previous
"""Multi-head attention (B=2, N=2048, C=768, H=12) on 8 TRN2 NeuronCores.

Sharding: core c = 4*b + g handles batch b (data parallel) and heads
3g..3g+2 (tensor parallel on H). Each core computes its 3 heads end-to-end
plus the partial projection with its 192 rows of w_proj; the host sums the
4 partials per batch and adds b_proj. No cross-device communication.

Per-core dataflow (f32 storage, fp32r TensorEngine compute):
  xt   = x[b].T                       [768, 2048]  (host layout choice)
  qkT  = [wq_h | wk_h].T @ xt chunks -> psum [128, 2048] = [qT_h; kT_h]
  kq   = partition-swapped copy        [kT_h; qT_h]  (SBUF->SBUF DMA)
  scoresT per 128-m-chunk: lhsT=kT chunk, rhs=qT  (K=64)
  expT = exp(0.125 * scoresT)          (ScalarE, [128,1024] granule)
  out_aug[65,nb] += v_aug_chunk.T @ expT  (v_aug = [v | 1] -> row 64 = colsum)
  recip broadcast via ones[1,64] matmul; outT = out_aug[0:64] * recip_bcast
  proj: out[nchunk,:] += outT_h[:, nchunk].T @ wp_h  (K=64 per head, accum)
"""

import ml_dtypes
import numpy as np

import concourse.bass as bass
import concourse.mybir as mybir
import concourse.tile as tile
from concourse import bacc
from concourse.bass_utils import run_bass_kernel_spmd

F32 = mybir.dt.float32
F32R = mybir.dt.float32r
BF16 = mybir.dt.bfloat16
EXP = mybir.ActivationFunctionType.Exp
MULT = mybir.AluOpType.mult

B, N, C = 2, 2048, 768
H = 12
D = 64
HPC = 3  # heads per core
KC = 6  # contraction chunks of 128 over C
NB = 1024  # n-block for attention stage
NSUB = NB // 512
MC = N // 128  # 16 m-chunks
NCH = N // 128  # 16 row chunks of output
SCALE = D ** -0.5

_NC_CACHE = None


def build_nc():
    nc = bacc.Bacc("TRN2", target_bir_lowering=False, debug=False, num_devices=8)
    xt = nc.declare_dram_parameter("xt", [C, N], BF16, isOutput=False)
    wqk = nc.declare_dram_parameter("wqk", [C, HPC * 128], BF16, isOutput=False)
    wv = nc.declare_dram_parameter("wv", [C, 256], BF16, isOutput=False)
    wp = nc.declare_dram_parameter("wp", [HPC * D, C], BF16, isOutput=False)
    out = nc.declare_dram_parameter("out", [N, C], BF16, isOutput=True)

    with tile.TileContext(nc) as tc:
        with tc.tile_pool(name="sb", bufs=1) as sb:
            # ---- load inputs -------------------------------------------------
            xtb = sb.tile([128, KC * N], BF16, tag="xtb")
            wqkb = sb.tile([128, KC * HPC * 128], BF16, tag="wqkb")
            wvb = sb.tile([128, KC * 256], BF16, tag="wvb")
            for kc in range(KC):
                nc.sync.dma_start(
                    wqkb[:, kc * HPC * 128 : (kc + 1) * HPC * 128],
                    wqk[kc * 128 : (kc + 1) * 128, :],
                )
                nc.sync.dma_start(
                    wvb[:, kc * 256 : (kc + 1) * 256],
                    wv[kc * 128 : (kc + 1) * 128, :],
                )
                nc.sync.dma_start(
                    xtb[:, kc * N : (kc + 1) * N], xt[kc * 128 : (kc + 1) * 128, :]
                )
            wp01 = sb.tile([128, C], BF16, tag="wp01")
            nc.sync.dma_start(wp01[:], wp[0:128, :])
            wp2 = sb.tile([64, C], BF16, tag="wp2")
            nc.sync.dma_start(wp2[:], wp[128 : HPC * D, :])

            # PE warmup: ~10us of junk matmuls to latch HAM to 2.4GHz
            junk = sb.tile([128, 512], BF16, tag="junk")
            nc.vector.memset(junk[:], 1.0)

            # constants
            ones_f = sb.tile([128, MC], F32, tag="ones_f")
            nc.vector.memset(ones_f[:], 1.0)

            # persistent activations
            v_sb = sb.tile([128, HPC * MC * 65], F32R, tag="v")
            v4 = v_sb.rearrange("p (h m w) -> p h m w", h=HPC, m=MC)
            for h in range(HPC):
                nc.vector.tensor_copy(v4[:, h, :, 64], ones_f[:, :])

            qk_sb = [sb.tile([128, N], BF16, tag=f"qk{h}", name=f"qk{h}") for h in range(HPC)]
            kq_sb = [sb.tile([128, N], BF16, tag=f"kq{h}", name=f"kq{h}") for h in range(HPC)]
            stk = sb.tile([128, N], BF16, tag="stk")
            outT1 = sb.tile([64, N], BF16, tag="outT1")
            outT2 = sb.tile([64, N], BF16, tag="outT2")

            # ---- phase 1: qkT per head; v natural ---------------------------
            with (
                tc.tile_pool(name="psum_qk", bufs=2, space="PSUM") as qkp,
                tc.tile_pool(name="psum_v", bufs=4, space="PSUM") as vp,
            ):
                for i in range(16):
                    psw = qkp.tile([128, 1024], F32, tag="psqk", name="psw")
                    nc.tensor.matmul(
                        psw[:, 0:512],
                        junk[:, 0:128],
                        junk[:],
                        start=True,
                        stop=True,
                    )
                for h in range(HPC):
                    for half in range(2):
                        hb = half * 1024
                        ps = qkp.tile([128, 1024], F32, tag="psqk")
                        for kc in range(KC):
                            for s in range(2):
                                nc.tensor.matmul(
                                    ps[:, s * 512 : (s + 1) * 512],
                                    wqkb[
                                        :,
                                        kc * HPC * 128
                                        + h * 128 : kc * HPC * 128
                                        + (h + 1) * 128,
                                    ],
                                    xtb[
                                        :,
                                        kc * N + hb + s * 512 : kc * N + hb + (s + 1) * 512,
                                    ],
                                    start=(kc == 0),
                                    stop=(kc == KC - 1),
                                )
                        if half == 0:
                            nc.vector.tensor_copy(qk_sb[h][:, hb : hb + 1024], ps[:])
                        else:
                            nc.scalar.copy(out=qk_sb[h][:, hb : hb + 1024], in_=ps[:])
                        nc.sync.dma_start(
                            kq_sb[h][0:64, hb : hb + 1024],
                            qk_sb[h][64:128, hb : hb + 1024],
                        )
                        nc.sync.dma_start(
                            kq_sb[h][64:128, hb : hb + 1024],
                            qk_sb[h][0:64, hb : hb + 1024],
                        )

                for m in range(MC):
                    psv = vp.tile([128, 256], F32, tag="psv")
                    for kc in range(KC):
                        nc.tensor.matmul(
                            psv[:],
                            xtb[:, kc * N + m * 128 : kc * N + (m + 1) * 128],
                            wvb[:, kc * 256 : (kc + 1) * 256],
                            start=(kc == 0),
                            stop=(kc == KC - 1),
                        )
                    nc.scalar.copy(
                        out=v4[:, :, m, 0:64],
                        in_=psv.rearrange("p (h d) -> p h d", h=4)[:, 0:HPC, :],
                    )

            # ---- phase 2: attention per head, per n-block --------------------
            with (
                tc.tile_pool(name="psum_sc", bufs=2, space="PSUM") as scp,
                tc.tile_pool(name="psum_oa", bufs=2, space="PSUM") as oap,
                tc.tile_pool(name="dram_r", bufs=2, space="DRAM") as drp,
            ):
                def oa_mms(oa, h, m):
                    exm = ex_tiles[m % 3]
                    for s in range(NSUB):
                        nc.tensor.matmul(
                            oa[:, s * 512 : (s + 1) * 512],
                            v_sb[:, (h * MC + m) * 65 : (h * MC + m + 1) * 65],
                            exm[:, s * 512 : (s + 1) * 512],
                            start=(m == 0),
                            stop=(m == MC - 1),
                        )

                for h in range(HPC):
                    for nb in range(N // NB):
                        oa = oap.tile([65, NB], F32, tag="oa")
                        ex_tiles = [None, None, None]
                        for m in range(MC):
                            sc = scp.tile([128, NB], F32, tag="sc")
                            # two 512-halves packed on the two PE array halves
                            nc.tensor.matmul(
                                sc[:, 0:512],
                                kq_sb[h][0:64, m * 128 : (m + 1) * 128],
                                qk_sb[h][0:64, nb * NB : nb * NB + 512],
                                start=True,
                                stop=True,
                                tile_position=(0, 0),
                            )
                            nc.tensor.matmul(
                                sc[:, 512:1024],
                                qk_sb[h][64:128, m * 128 : (m + 1) * 128],
                                kq_sb[h][64:128, nb * NB + 512 : nb * NB + 1024],
                                start=True,
                                stop=True,
                                tile_position=(64, 0),
                            )
                            ex = sb.tile([128, NB], F32R, tag="ex", bufs=3)
                            nc.scalar.activation(ex[:], sc[:], EXP, scale=SCALE)
                            ex_tiles[m % 3] = ex
                            if m >= 1:
                                oa_mms(oa, h, m - 1)
                        oa_mms(oa, h, MC - 1)
                        # softmax normalization
                        cs = sb.tile([1, NB], F32, tag="cs", bufs=2)
                        nc.vector.tensor_copy(cs[:], oa[64:65, :])
                        rf = sb.tile([1, NB], F32, tag="rf", bufs=2)
                        nc.vector.reciprocal_approx_fast(out=rf[:], in_=cs[:])
                        rfd = drp.tile([1, NB], F32, tag="rfd", bufs=2)
                        nc.sync.dma_start(rfd[:], rf[:])
                        rbs = sb.tile([64, NB], F32, tag="rbs", bufs=2)
                        nc.sync.dma_start(rbs[:], rfd[:].partition_broadcast(64))
                        if h == 0:
                            mdst = stk[0:64, nb * NB : (nb + 1) * NB]
                        elif h == 1:
                            mdst = outT1[0:64, nb * NB : (nb + 1) * NB]
                        else:
                            mdst = outT2[0:64, nb * NB : (nb + 1) * NB]
                        nc.vector.tensor_tensor(
                            out=mdst,
                            in0=oa[0:64, :],
                            in1=rbs[:],
                            op=MULT,
                        )
                        if h == 1:
                            nc.sync.dma_start(
                                stk[64:128, nb * NB : (nb + 1) * NB],
                                outT1[0:64, nb * NB : (nb + 1) * NB],
                            )

            # ---- phase 3: projection (partial over this core's 192 chans) ---
            with tc.tile_pool(name="psum_pj", bufs=2, space="PSUM") as pjp:
                for m in range(NCH):
                    pp = pjp.tile([128, C], F32, tag="pp")
                    for s, w in ((0, 512), (512, 256)):
                        nc.tensor.matmul(
                            pp[:, s : s + w],
                            stk[:, m * 128 : (m + 1) * 128],
                            wp01[:, s : s + w],
                            start=True,
                            stop=False,
                        )
                    for s, w in ((0, 512), (512, 256)):
                        nc.tensor.matmul(
                            pp[:, s : s + w],
                            outT2[0:64, m * 128 : (m + 1) * 128],
                            wp2[:, s : s + w],
                            start=False,
                            stop=True,
                        )
                    ob = sb.tile([128, C], BF16, tag="ob", bufs=4)
                    nc.vector.tensor_copy(ob[:, 0:384], pp[:, 0:384])
                    nc.scalar.copy(out=ob[:, 384:768], in_=pp[:, 384:768])
                    nc.sync.dma_start(out[m * 128 : (m + 1) * 128, 0:384], ob[:, 0:384])
                    nc.sync.dma_start(out[m * 128 : (m + 1) * 128, 384:768], ob[:, 384:768])

    nc.compile()
    return nc


def get_nc():
    global _NC_CACHE
    if _NC_CACHE is None:
        _NC_CACHE = build_nc()
    return _NC_CACHE


def make_in_maps(x, w_qkv, w_proj):
    """Shard inputs for the 8 cores: core c = 4*b + g."""
    in_maps = []
    for c in range(8):
        b, g = divmod(c, 4)
        heads = [3 * g + h for h in range(HPC)]
        xt = np.ascontiguousarray(x[b].T).astype(ml_dtypes.bfloat16)
        wqk = np.empty((C, HPC * 128), dtype=ml_dtypes.bfloat16)
        wv = np.zeros((C, 256), dtype=ml_dtypes.bfloat16)
        for i, hh in enumerate(heads):
            wqk[:, i * 128 : i * 128 + 64] = w_qkv[:, hh * D : (hh + 1) * D]
            wqk[:, i * 128 + 64 : i * 128 + 128] = w_qkv[
                :, C + hh * D : C + (hh + 1) * D
            ]
            wv[:, i * D : (i + 1) * D] = w_qkv[:, 2 * C + hh * D : 2 * C + (hh + 1) * D]
        wp = np.ascontiguousarray(
            w_proj[g * HPC * D : (g + 1) * HPC * D, :]
        ).astype(ml_dtypes.bfloat16)
        in_maps.append(
            {"xt": xt, "wqk": np.ascontiguousarray(wqk), "wv": wv, "wp": wp}
        )
    return in_maps


def run(x, w_qkv, w_proj, b_proj, trace=False):
    nc = get_nc()
    in_maps = make_in_maps(x, w_qkv, w_proj)
    res = run_bass_kernel_spmd(nc, in_maps, core_ids=list(range(8)), trace=trace)
    out = np.empty((B, N, C), dtype=np.float32)
    for b in range(B):
        acc = res.results[4 * b]["out"].astype(np.float32)
        for g in range(1, 4):
            acc = acc + res.results[4 * b + g]["out"]
        out[b] = acc + b_proj[None, :].astype(np.float32)
    return out, res


def kernel(x, w_qkv, w_proj, b_proj):
    out, _ = run(
        np.asarray(x), np.asarray(w_qkv), np.asarray(w_proj), np.asarray(b_proj)
    )
    return out
